# revision 21
# baseline (speedup 1.0000x reference)
"""Differential attention on 8 trn2 NeuronCores.

Sharding: data-parallel over batch (2 groups of 4 cores) x tensor-parallel
over heads (4 heads/core). Each core computes its head-group's qkv
projections, dual softmax attention, and a partial output projection over
its 256 channels, plus the per-token sum-of-squares needed for the RMSNorm.
The host sums the 4 partial projections per batch, applies the RMS scale
(which commutes with the channel contraction) and the bias.

All matmuls run as float32r (full-rate fp32 streaming on the PE).
Layouts are feature-major ([feature, token]) so softmax rowsums ride the
attention@V matmul via a ones-augmented V, avoiding cross-partition
reductions.

Schedule: one fully software-pipelined pass. The ACT engine does nothing
but the 256 softmax exps (its 266us is one of the two roofline terms; the
other is the PE's 302us of matmul streaming), so every other op lives on
DVE/GPSIMD. The V/QK projections, output projection and sum-of-squares
matmuls are interleaved one-at-a-time into the attention group loop as
background fillers, keeping the PE dense and at max p-state while the
attention stream stays exp-throughput-limited.
"""
import sys

sys.path.insert(0, "/opt/trn_rl_repo")

import numpy as np

import concourse.bass as bass
import concourse.mybir as mybir
import concourse.tile as tile
from concourse import bacc, bass_utils
from concourse.bass_interp import get_hw_module

F32 = mybir.dt.float32
F32R = mybir.dt.float32r
AF = mybir.ActivationFunctionType
OP = mybir.AluOpType
AX = mybir.AxisListType

B, N, DIM = 2, 2048, 1024
H, HD = 16, 64
HPC = 4          # heads per core
CH = HPC * HD    # channels per core (256)
SCALE = HD ** -0.5
EPS = 1e-5
NT = N // 128    # 16 token tiles
QC = N // 512    # 4 query chunks
CT = DIM // 128  # 8 contraction tiles


def r(ap):
    return ap.bitcast(F32R)


def build_program(nc, debug=False):
    xt = nc.dram_tensor("xt", [DIM, N], F32, kind="ExternalInput").ap()
    wqk = nc.dram_tensor("wqk", [DIM, 8 * 128], F32, kind="ExternalInput").ap()
    wv = nc.dram_tensor("wv", [DIM, CH], F32, kind="ExternalInput").ap()
    wp = nc.dram_tensor("wp", [CH, DIM], F32, kind="ExternalInput").ap()
    lam = nc.dram_tensor("lam", [1, 4 * HD], F32, kind="ExternalInput").ap()
    out = nc.dram_tensor("out", [DIM, N], F32, kind="ExternalOutput").ap()
    ssq = nc.dram_tensor("ssq", [1, N], F32, kind="ExternalOutput").ap()

    with tile.TileContext(nc) as tc:
        with (
            nc.allow_low_precision(reason="float32r matmul operand rounding is intentional"),
            tc.tile_pool(name="persist", bufs=1) as pp,
            tc.tile_pool(name="qkp", bufs=4) as qkpool,
            tc.tile_pool(name="wqkp", bufs=4) as wqkpool,
            tc.tile_pool(name="upool", bufs=3) as upool,
            tc.tile_pool(name="cpool", bufs=2) as cpool,
            tc.tile_pool(name="rpool", bufs=1) as rpool,
            tc.tile_pool(name="tpool", bufs=1) as tpool,
            tc.tile_pool(name="obuf", bufs=2) as obuf,
            tc.tile_pool(name="sqpool", bufs=2) as sqpool,
            tc.tile_pool(name="psA", bufs=2, space="PSUM") as psA,
        ):
            # ---- input DMAs: small weight tensors first (they gate the very
            # first matmuls), then x in fine-grained token chunks ----
            wv_sb = pp.tile([128, CT, CH], F32R, tag="wv")
            wv_r = wv.rearrange("(t p) f -> p t f", p=128)
            for c in range(4):
                nc.sync.dma_start(
                    wv_sb[:, 2 * c:2 * c + 2, :], wv_r[:, 2 * c:2 * c + 2, :].bitcast(F32R)
                )

            wqk_r = wqk.rearrange("(t p) f -> p t f", p=128)

            def load_wqk(ft):
                w = wqkpool.tile([128, CT, 128], F32R, tag="wqk", name=f"wqk{ft}")
                for c in range(2):
                    nc.sync.dma_start(
                        w[:, 4 * c:4 * c + 4, :],
                        wqk_r[:, 4 * c:4 * c + 4, ft * 128:(ft + 1) * 128].bitcast(F32R),
                    )
                return w

            # head 0 + head 1 weights up front; later heads prefetched in-loop
            w_q = [None] * HPC
            w_k = [None] * HPC
            w_q[0] = load_wqk(0)
            w_k[0] = load_wqk(4)

            lam_sb = pp.tile([1, 4 * HD], F32, tag="lam")
            nc.sync.dma_start(lam_sb[:], lam[:])

            x_sb = pp.tile([128, CT, N], F32R, tag="x")
            xt_r = xt.rearrange("(t p) n -> p t n", p=128)
            for c in range(16):
                lo, hi = c * 128, (c + 1) * 128
                nc.sync.dma_start(x_sb[:, :, lo:hi], xt_r[:, :, lo:hi].bitcast(F32R))

            w_q[1] = load_wqk(1)
            w_k[1] = load_wqk(5)

            wp_sb = pp.tile([128, 2, DIM], F32R, tag="wp")
            nc.sync.dma_start(wp_sb[:], wp.rearrange("(t p) o -> p t o", p=128).bitcast(F32R))

            # ---- constants / lambda ----
            ones128 = pp.tile([128, 1], F32R, tag="ones128")
            nc.vector.memset(ones128.bitcast(mybir.dt.uint32)[:], 0x3F800000)
            lprod = pp.tile([1, 2 * HD], F32, tag="lprod")
            nc.vector.tensor_mul(lprod[:, 0:HD], lam_sb[:, 0:HD], lam_sb[:, HD:2 * HD])
            nc.vector.tensor_mul(
                lprod[:, HD:2 * HD], lam_sb[:, 2 * HD:3 * HD], lam_sb[:, 3 * HD:4 * HD]
            )
            lsum = pp.tile([1, 2], F32, tag="lsum")
            nc.vector.reduce_sum(lsum[:, 0:1], lprod[:, 0:HD], axis=AX.X)
            nc.vector.reduce_sum(lsum[:, 1:2], lprod[:, HD:2 * HD], axis=AX.X)
            lexp = pp.tile([1, 2], F32, tag="lexp")
            nc.scalar.activation(lexp[:], lsum[:], AF.Exp)
            negl = pp.tile([1, 1], F32, tag="negl")
            # -lambda_full = exp(sum lq2*lk2) - exp(sum lq1*lk1) - 0.8
            nc.vector.tensor_sub(negl[:], lexp[:, 1:2], lexp[:, 0:1])
            nc.vector.tensor_scalar_add(negl[:], negl[:], -0.8)

            # ---- persistent big tiles ----
            vaug = pp.tile([128, HPC, NT, HD + 1], F32, tag="vaug")
            nc.vector.memset(vaug[:, :, :, HD:HD + 1].bitcast(mybir.dt.uint32), 0x3F800000)
            o_t = [pp.tile([128, N], F32R, tag=f"obig{i}", name=f"obig{i}") for i in range(2)]

            # ---- production pieces (emitted interleaved with attention) ----
            def v_nt(nt):
                ps = psA.tile([128, 512], F32, tag="psqk", name=f"psv{nt}")
                for ct in range(CT):
                    nc.tensor.matmul(
                        ps[:, 0:CH],
                        lhsT=r(x_sb[:, ct, nt * 128:(nt + 1) * 128]),
                        rhs=r(wv_sb[:, ct, :]),
                        start=(ct == 0),
                        stop=(ct == CT - 1),
                    )
                nc.vector.tensor_copy(
                    r(vaug[:, :, nt, 0:HD]),
                    r(ps[:, 0:CH]).rearrange("p (h d) -> p h d", d=HD),
                )

            def qk_piece(w_sb, dst, qc):
                ps = psA.tile([128, 512], F32, tag="psqk")
                for ct in range(CT):
                    nc.tensor.matmul(
                        ps[:],
                        lhsT=r(w_sb[:, ct, :]),
                        rhs=r(x_sb[:, ct, qc * 512:(qc + 1) * 512]),
                        start=(ct == 0),
                        stop=(ct == CT - 1),
                    )
                nc.vector.tensor_copy(r(dst[:, qc * 512:(qc + 1) * 512]), r(ps[:]))

            def qk_steps(wi, dst, qc):
                # one QK projection chunk split into two 4-matmul background
                # steps so interleaving it never starves the exp pipeline
                cell = {}

                def a(wi=wi, dst=dst, qc=qc):
                    ps = psA.tile([128, 512], F32, tag="psqk")
                    cell["ps"] = ps
                    for ct in range(4):
                        nc.tensor.matmul(
                            ps[:],
                            lhsT=r(w_sb_at(wi)[:, ct, :]),
                            rhs=r(x_sb[:, ct, qc * 512:(qc + 1) * 512]),
                            start=(ct == 0),
                            stop=False,
                        )

                def b(wi=wi, dst=dst, qc=qc):
                    ps = cell["ps"]
                    for ct in range(4, CT):
                        nc.tensor.matmul(
                            ps[:],
                            lhsT=r(w_sb_at(wi)[:, ct, :]),
                            rhs=r(x_sb[:, ct, qc * 512:(qc + 1) * 512]),
                            start=False,
                            stop=(ct == CT - 1),
                        )
                    nc.vector.tensor_copy(r(dst[:, qc * 512:(qc + 1) * 512]), r(ps[:]))

                return [a, b]

            def proj_piece(nch, ot):
                ps = psA.tile([128, 512], F32, tag="psqk")
                for t in range(2):
                    nc.tensor.matmul(
                        ps[:],
                        lhsT=r(wp_sb[:, t, ot * 128:(ot + 1) * 128]),
                        rhs=r(o_t[t][:, nch * 512:(nch + 1) * 512]),
                        start=(t == 0),
                        stop=(t == 1),
                    )
                ob = obuf.tile([128, 512], F32, tag="ob")
                nc.vector.tensor_copy(ob[:], ps[:])
                nc.sync.dma_start(
                    out[ot * 128:(ot + 1) * 128, nch * 512:(nch + 1) * 512], ob[:]
                )

            qk_tiles = {}
            for h in range(HPC):
                qk_tiles[h] = (
                    qkpool.tile([128, N], F32, tag="qk", name=f"tq{h}"),
                    qkpool.tile([128, N], F32, tag="qk", name=f"tk{h}"),
                )

            def combine(h, qc, o1, o2):
                # stage the two rowsum rows at partition 0 (the custom-DVE
                # approx reciprocal mis-reads partition-offset inputs on HW),
                # evacuate psum (frees the po banks for the next qc), then
                # normalize: 1/r via the fast approx reciprocal, fold -lambda
                # into the second half, replicate across the 64 hd-rows on
                # gpsimd, scale + add into the head-group rows.
                rs = rpool.tile([1, 1024], F32, tag="rs", name=f"rs_{h}_{qc}")
                nc.vector.tensor_copy(rs[:, 0:512], o1[HD:HD + 1, :])
                nc.vector.tensor_copy(rs[:, 512:1024], o2[HD:HD + 1, :])
                o12 = cpool.tile([HD, 1024], F32, tag="o12", name=f"o12_{h}_{qc}")
                nc.vector.tensor_copy(o12[:, 0:512], o1[0:HD, :])
                nc.vector.tensor_copy(o12[:, 512:1024], o2[0:HD, :])
                rr = rpool.tile([1, 1024], F32, tag="rr", name=f"rr_{h}_{qc}")
                nc.vector.reciprocal_approx_fast(out=rr[:], in_=rs[:])
                nc.vector.tensor_scalar_mul(rr[:, 512:1024], rr[:, 512:1024], negl[:])
                rep = tpool.tile([HD, 1024], F32, tag="repb", name=f"repb_{h}_{qc}")
                nc.gpsimd.partition_broadcast(rep[:], rr[:])
                t12 = tpool.tile([HD, 1024], F32, tag="t12")
                nc.vector.tensor_mul(t12[:], o12[0:HD, :], rep[0:HD, 0:1024])
                nc.vector.tensor_add(
                    o_t[h // 2][
                        (h % 2) * 64:(h % 2) * 64 + 64,
                        qc * 512:(qc + 1) * 512,
                    ],
                    t12[:, 0:512],
                    t12[:, 512:1024],
                )

            def w_sb_at(wi):
                kind, hh = wi
                return (w_q if kind == "q" else w_k)[hh]

            bkq = []  # FIFO of background PE step closures

            with (
                tc.tile_pool(name="slots", bufs=2, space="PSUM") as slots,
                tc.tile_pool(name="po", bufs=2, space="PSUM") as po,
            ):
                # pending attn@V pairs: emitted two groups late so the PE's
                # av matmuls trail the exp by two full groups of slack.
                pend = []

                def emit_av(h, o1, o2, kt, u):
                    nc.tensor.matmul(
                        o1[:],
                        lhsT=r(vaug[:, h, kt, :]),
                        rhs=r(u[:, 0:512]),
                        start=(kt == 0),
                        stop=(kt == NT - 1),
                    )
                    nc.tensor.matmul(
                        o2[:],
                        lhsT=r(vaug[:, h, kt, :]),
                        rhs=r(u[:, 512:1024]),
                        start=(kt == 0),
                        stop=(kt == NT - 1),
                    )

                def group(h, tq, tk, qc, kt, o1, o2):
                    sl = slots.tile([128, 1024], F32, tag="slot")
                    for term in range(2):
                        rb = term * 64
                        nc.tensor.matmul(
                            sl[:, term * 512:(term + 1) * 512],
                            lhsT=r(tk[rb:rb + 64, kt * 128:(kt + 1) * 128]),
                            rhs=r(tq[rb:rb + 64, qc * 512:(qc + 1) * 512]),
                            start=True,
                            stop=True,
                        )
                    u = upool.tile([128, 1024], F32R, tag="u")
                    nc.scalar.activation(u[:], sl[:], AF.Exp, scale=SCALE)
                    if len(pend) == 2:
                        emit_av(*pend.pop(0))
                    pend.append((h, o1, o2, kt, u))

                # -- head 0, qc 0: interleave the V/K production with the
                # first attention groups (each group g consumes key-block
                # kt=g, whose v/k tiles are produced a few groups ahead).
                tq0, tk0 = qk_tiles[0]
                for nt in range(4):
                    v_nt(nt)
                qk_piece(w_q[0], tq0, 0)
                qk_piece(w_k[0], tk0, 0)
                q0fill = [
                    [lambda: v_nt(4), lambda: v_nt(5)],
                    [lambda: qk_piece(w_k[0], tk0, 1)],
                    [lambda: v_nt(6), lambda: v_nt(7)],
                    [lambda: qk_piece(w_k[0], tk0, 2)],
                    [lambda: v_nt(8), lambda: v_nt(9)],
                    [lambda: v_nt(10), lambda: v_nt(11)],
                    [lambda: qk_piece(w_k[0], tk0, 3)],
                    [lambda: v_nt(12), lambda: v_nt(13)],
                    [lambda: v_nt(14), lambda: v_nt(15)],
                    [lambda: qk_piece(w_q[0], tq0, 1)],
                    [lambda: qk_piece(w_q[0], tq0, 2)],
                    [lambda: qk_piece(w_q[0], tq0, 3)],
                    [], [], [], [],
                ]

                for h in range(HPC):
                    tq, tk = qk_tiles[h]
                    if h == 1:
                        w_q[3] = load_wqk(3)
                        w_k[3] = load_wqk(7)
                    if h < 3:
                        nh = h + 1
                        tqn, tkn = qk_tiles[nh]
                        for qc in range(QC):
                            bkq += qk_steps(("q", nh), tqn, qc)
                            bkq += qk_steps(("k", nh), tkn, qc)
                    for qc in range(QC):
                        if h == 0 and qc == 1:
                            w_q[2] = load_wqk(2)
                            w_k[2] = load_wqk(6)
                        o1 = po.tile([HD + 1, 512], F32, tag="oacc")
                        o2 = po.tile([HD + 1, 512], F32, tag="oacc")
                        first = (h == 0 and qc == 0)
                        stride = max(1, ((QC - qc) * NT) // max(len(bkq), 1))
                        for kt in range(NT):
                            group(h, tq, tk, qc, kt, o1, o2)
                            if first:
                                for f in q0fill[kt]:
                                    f()
                            elif bkq and kt % stride == stride - 1:
                                bkq.pop(0)()
                        while pend:
                            emit_av(*pend.pop(0))
                        combine(h, qc, o1, o2)
                        if h == 3 and qc < QC - 1:
                            # output projection for this q-chunk (all heads'
                            # combines for it are now done)
                            for ot in range(8):
                                bkq.append(lambda nch=qc, ot=ot: proj_piece(nch, ot))
                    while bkq:
                        bkq.pop(0)()

            # ---- tail: last proj chunk + sum-of-squares ----
            with tc.tile_pool(name="psS", bufs=1, space="PSUM") as psS:
                for ot in range(8):
                    proj_piece(3, ot)
                ssq_ps = psS.tile([1, N], F32, tag="ssqp")
                for c4 in range(QC):
                    for t in range(2):
                        sq = sqpool.tile([128, 512], F32R, tag="sq", bufs=1)
                        nc.scalar.activation(
                            sq[:], o_t[t][:, c4 * 512:(c4 + 1) * 512], AF.Square
                        )
                        nc.tensor.matmul(
                            ssq_ps[:, c4 * 512:(c4 + 1) * 512],
                            lhsT=r(ones128[:]),
                            rhs=r(sq[:]),
                            start=(t == 0),
                            stop=(t == 1),
                        )
                for c4 in range(QC):
                    ssq_sb = sqpool.tile([1, 512], F32, tag="ssqs", bufs=1)
                    nc.vector.tensor_copy(ssq_sb[:], ssq_ps[:, c4 * 512:(c4 + 1) * 512])
                    nc.sync.dma_start(ssq[:, c4 * 512:(c4 + 1) * 512], ssq_sb[:])
            if debug:
                d_ot0 = nc.dram_tensor("d_ot0", [128, N], F32, kind="ExternalOutput").ap()
                d_ot1 = nc.dram_tensor("d_ot1", [128, N], F32, kind="ExternalOutput").ap()
                d_va = nc.dram_tensor(
                    "d_va", [128, HPC * NT * (HD + 1)], F32, kind="ExternalOutput"
                ).ap()
                nc.sync.dma_start(d_ot0[:], o_t[0].bitcast(F32)[:])
                nc.sync.dma_start(d_ot1[:], o_t[1].bitcast(F32)[:])
                nc.sync.dma_start(
                    d_va[:], vaug.rearrange("p a b c -> p (a b c)")[:]
                )
    return nc


_CACHE = {}


def get_nc():
    if "nc" not in _CACHE:
        nc = bacc.Bacc(
            "TRN2", target_bir_lowering=False, debug=False, enable_asserts=False
        )
        build_program(nc)
        nc.compile()
        nc.m = get_hw_module(nc.m)
        _CACHE["nc"] = nc
    return _CACHE["nc"]


def make_in_maps(x, qkv_w, proj_w, lambda_q1, lambda_k1, lambda_q2, lambda_k2):
    x = np.asarray(x, np.float32)
    qkv_w = np.asarray(qkv_w, np.float32)
    proj_w = np.asarray(proj_w, np.float32)
    lamv = np.concatenate(
        [np.asarray(a, np.float32) for a in (lambda_q1, lambda_k1, lambda_q2, lambda_k2)]
    )[None, :]
    in_maps = []
    for core in range(8):
        b, hg = core // 4, core % 4
        h0 = hg * HPC
        rows = []
        for h in range(h0, h0 + HPC):
            rows.append(qkv_w[0 * DIM + h * HD:0 * DIM + (h + 1) * HD])
            rows.append(qkv_w[1 * DIM + h * HD:1 * DIM + (h + 1) * HD])
        for h in range(h0, h0 + HPC):
            rows.append(qkv_w[2 * DIM + h * HD:2 * DIM + (h + 1) * HD])
            rows.append(qkv_w[3 * DIM + h * HD:3 * DIM + (h + 1) * HD])
        wqk_np = np.ascontiguousarray(np.concatenate(rows, 0).T)
        wv_np = np.ascontiguousarray(
            np.concatenate(
                [qkv_w[4 * DIM + h * HD:4 * DIM + (h + 1) * HD] for h in range(h0, h0 + HPC)],
                0,
            ).T
        )
        wp_np = np.ascontiguousarray(proj_w[:, h0 * HD:(h0 + HPC) * HD].T)
        in_maps.append(
            {
                "xt": np.ascontiguousarray(x[b].T),
                "wqk": wqk_np,
                "wv": wv_np,
                "wp": wp_np,
                "lam": np.ascontiguousarray(lamv),
            }
        )
    return in_maps


def combine(results, proj_b):
    proj_b = np.asarray(proj_b, np.float32)
    y = np.empty((B, N, DIM), np.float32)
    for b in range(B):
        acc = np.zeros((DIM, N), np.float64)
        sq = np.zeros(N, np.float64)
        for g in range(4):
            rr = results[b * 4 + g]
            acc += rr["out"].astype(np.float64)
            sq += rr["ssq"][0].astype(np.float64)
        s = 0.2 / np.sqrt(sq / DIM + EPS)
        y[b] = (acc.T * s[:, None] + proj_b).astype(np.float32)
    return y


def kernel(x, qkv_w, proj_w, proj_b, lambda_q1, lambda_k1, lambda_q2, lambda_k2):
    nc = get_nc()
    in_maps = make_in_maps(
        x, qkv_w, proj_w, lambda_q1, lambda_k1, lambda_q2, lambda_k2
    )
    res = bass_utils.run_bass_kernel_spmd(nc, in_maps, core_ids=list(range(8)))
    return combine(res.results, proj_b)


# revision 23
# speedup vs baseline: 1.0064x; 1.0064x over previous
"""Differential attention on 8 trn2 NeuronCores.

Sharding: data-parallel over batch (2 groups of 4 cores) x tensor-parallel
over heads (4 heads/core). Each core computes its head-group's qkv
projections, dual softmax attention, and a partial output projection over
its 256 channels, plus the per-token sum-of-squares needed for the RMSNorm.
The host sums the 4 partial projections per batch, applies the RMS scale
(which commutes with the channel contraction) and the bias.

All matmuls run as float32r (full-rate fp32 streaming on the PE).
Layouts are feature-major ([feature, token]) so softmax rowsums ride the
attention@V matmul via a ones-augmented V, avoiding cross-partition
reductions.

Schedule: one fully software-pipelined pass. The ACT engine does nothing
but the 256 softmax exps (its 266us is one of the two roofline terms; the
other is the PE's 302us of matmul streaming), so every other op lives on
DVE/GPSIMD. The V/QK projections, output projection and sum-of-squares
matmuls are interleaved one-at-a-time into the attention group loop as
background fillers, keeping the PE dense and at max p-state while the
attention stream stays exp-throughput-limited.
"""
import sys

sys.path.insert(0, "/opt/trn_rl_repo")

import numpy as np

import concourse.bass as bass
import concourse.mybir as mybir
import concourse.tile as tile
from concourse import bacc, bass_utils
from concourse.bass_interp import get_hw_module

F32 = mybir.dt.float32
F32R = mybir.dt.float32r
AF = mybir.ActivationFunctionType
OP = mybir.AluOpType
AX = mybir.AxisListType

B, N, DIM = 2, 2048, 1024
H, HD = 16, 64
HPC = 4          # heads per core
CH = HPC * HD    # channels per core (256)
SCALE = HD ** -0.5
EPS = 1e-5
NT = N // 128    # 16 token tiles
QC = N // 512    # 4 query chunks
CT = DIM // 128  # 8 contraction tiles


def r(ap):
    return ap.bitcast(F32R)


def build_program(nc, debug=False):
    xt = nc.dram_tensor("xt", [DIM, N], F32, kind="ExternalInput").ap()
    wqk = nc.dram_tensor("wqk", [DIM, 8 * 128], F32, kind="ExternalInput").ap()
    wv = nc.dram_tensor("wv", [DIM, CH], F32, kind="ExternalInput").ap()
    wp = nc.dram_tensor("wp", [CH, DIM], F32, kind="ExternalInput").ap()
    lam = nc.dram_tensor("lam", [1, 4 * HD], F32, kind="ExternalInput").ap()
    out = nc.dram_tensor("out", [DIM, N], F32, kind="ExternalOutput").ap()
    ssq = nc.dram_tensor("ssq", [1, N], F32, kind="ExternalOutput").ap()

    with tile.TileContext(nc) as tc:
        with (
            nc.allow_low_precision(reason="float32r matmul operand rounding is intentional"),
            tc.tile_pool(name="persist", bufs=1) as pp,
            tc.tile_pool(name="qkp", bufs=4) as qkpool,
            tc.tile_pool(name="wqkp", bufs=4) as wqkpool,
            tc.tile_pool(name="upool", bufs=4) as upool,
            tc.tile_pool(name="cpool", bufs=1) as cpool,
            tc.tile_pool(name="rpool", bufs=1) as rpool,
            tc.tile_pool(name="tpool", bufs=1) as tpool,
            tc.tile_pool(name="obuf", bufs=2) as obuf,
            tc.tile_pool(name="sqpool", bufs=2) as sqpool,
            tc.tile_pool(name="psA", bufs=2, space="PSUM") as psA,
        ):
            # ---- input DMAs: small weight tensors first (they gate the very
            # first matmuls), then x in fine-grained token chunks ----
            wv_sb = pp.tile([128, CT, CH], F32R, tag="wv")
            wv_r = wv.rearrange("(t p) f -> p t f", p=128)
            for c in range(4):
                nc.sync.dma_start(
                    wv_sb[:, 2 * c:2 * c + 2, :], wv_r[:, 2 * c:2 * c + 2, :].bitcast(F32R)
                )

            wqk_r = wqk.rearrange("(t p) f -> p t f", p=128)

            def load_wqk(ft):
                w = wqkpool.tile([128, CT, 128], F32R, tag="wqk", name=f"wqk{ft}")
                for c in range(2):
                    nc.sync.dma_start(
                        w[:, 4 * c:4 * c + 4, :],
                        wqk_r[:, 4 * c:4 * c + 4, ft * 128:(ft + 1) * 128].bitcast(F32R),
                    )
                return w

            # head 0 + head 1 weights up front; later heads prefetched in-loop
            w_q = [None] * HPC
            w_k = [None] * HPC
            w_q[0] = load_wqk(0)
            w_k[0] = load_wqk(4)

            lam_sb = pp.tile([1, 4 * HD], F32, tag="lam")
            nc.sync.dma_start(lam_sb[:], lam[:])

            x_sb = pp.tile([128, CT, N], F32R, tag="x")
            xt_r = xt.rearrange("(t p) n -> p t n", p=128)
            for c in range(16):
                lo, hi = c * 128, (c + 1) * 128
                nc.sync.dma_start(x_sb[:, :, lo:hi], xt_r[:, :, lo:hi].bitcast(F32R))

            w_q[1] = load_wqk(1)
            w_k[1] = load_wqk(5)

            wp_sb = pp.tile([128, 2, DIM], F32R, tag="wp")
            nc.sync.dma_start(wp_sb[:], wp.rearrange("(t p) o -> p t o", p=128).bitcast(F32R))

            # ---- constants / lambda ----
            ones128 = pp.tile([128, 1], F32R, tag="ones128")
            nc.vector.memset(ones128.bitcast(mybir.dt.uint32)[:], 0x3F800000)
            lprod = pp.tile([1, 2 * HD], F32, tag="lprod")
            nc.vector.tensor_mul(lprod[:, 0:HD], lam_sb[:, 0:HD], lam_sb[:, HD:2 * HD])
            nc.vector.tensor_mul(
                lprod[:, HD:2 * HD], lam_sb[:, 2 * HD:3 * HD], lam_sb[:, 3 * HD:4 * HD]
            )
            lsum = pp.tile([1, 2], F32, tag="lsum")
            nc.vector.reduce_sum(lsum[:, 0:1], lprod[:, 0:HD], axis=AX.X)
            nc.vector.reduce_sum(lsum[:, 1:2], lprod[:, HD:2 * HD], axis=AX.X)
            lexp = pp.tile([1, 2], F32, tag="lexp")
            nc.scalar.activation(lexp[:], lsum[:], AF.Exp)
            negl = pp.tile([1, 1], F32, tag="negl")
            # -lambda_full = exp(sum lq2*lk2) - exp(sum lq1*lk1) - 0.8
            nc.vector.tensor_sub(negl[:], lexp[:, 1:2], lexp[:, 0:1])
            nc.vector.tensor_scalar_add(negl[:], negl[:], -0.8)

            # ---- persistent big tiles ----
            vaug = pp.tile([128, HPC, NT, HD + 1], F32, tag="vaug")
            nc.vector.memset(vaug[:, :, :, HD:HD + 1].bitcast(mybir.dt.uint32), 0x3F800000)
            o_t = [pp.tile([128, N], F32R, tag=f"obig{i}", name=f"obig{i}") for i in range(2)]

            # ---- production pieces (emitted interleaved with attention) ----
            def v_nt(nt):
                ps = psA.tile([128, 512], F32, tag="psqk", name=f"psv{nt}")
                for ct in range(CT):
                    nc.tensor.matmul(
                        ps[:, 0:CH],
                        lhsT=r(x_sb[:, ct, nt * 128:(nt + 1) * 128]),
                        rhs=r(wv_sb[:, ct, :]),
                        start=(ct == 0),
                        stop=(ct == CT - 1),
                    )
                nc.vector.tensor_copy(
                    r(vaug[:, :, nt, 0:HD]),
                    r(ps[:, 0:CH]).rearrange("p (h d) -> p h d", d=HD),
                )

            def qk_piece(w_sb, dst, qc):
                ps = psA.tile([128, 512], F32, tag="psqk")
                for ct in range(CT):
                    nc.tensor.matmul(
                        ps[:],
                        lhsT=r(w_sb[:, ct, :]),
                        rhs=r(x_sb[:, ct, qc * 512:(qc + 1) * 512]),
                        start=(ct == 0),
                        stop=(ct == CT - 1),
                    )
                nc.vector.tensor_copy(r(dst[:, qc * 512:(qc + 1) * 512]), r(ps[:]))

            def qk_steps(wi, dst, qc):
                # one QK projection chunk split into two 4-matmul background
                # steps so interleaving it never starves the exp pipeline
                cell = {}

                def a(wi=wi, dst=dst, qc=qc):
                    ps = psA.tile([128, 512], F32, tag="psqk")
                    cell["ps"] = ps
                    for ct in range(4):
                        nc.tensor.matmul(
                            ps[:],
                            lhsT=r(w_sb_at(wi)[:, ct, :]),
                            rhs=r(x_sb[:, ct, qc * 512:(qc + 1) * 512]),
                            start=(ct == 0),
                            stop=False,
                        )

                def b(wi=wi, dst=dst, qc=qc):
                    ps = cell["ps"]
                    for ct in range(4, CT):
                        nc.tensor.matmul(
                            ps[:],
                            lhsT=r(w_sb_at(wi)[:, ct, :]),
                            rhs=r(x_sb[:, ct, qc * 512:(qc + 1) * 512]),
                            start=False,
                            stop=(ct == CT - 1),
                        )
                    nc.vector.tensor_copy(r(dst[:, qc * 512:(qc + 1) * 512]), r(ps[:]))

                return [a, b]

            def proj_piece(nch, ot):
                ps = psA.tile([128, 512], F32, tag="psqk")
                for t in range(2):
                    nc.tensor.matmul(
                        ps[:],
                        lhsT=r(wp_sb[:, t, ot * 128:(ot + 1) * 128]),
                        rhs=r(o_t[t][:, nch * 512:(nch + 1) * 512]),
                        start=(t == 0),
                        stop=(t == 1),
                    )
                ob = obuf.tile([128, 512], F32, tag="ob")
                nc.vector.tensor_copy(ob[:], ps[:])
                nc.sync.dma_start(
                    out[ot * 128:(ot + 1) * 128, nch * 512:(nch + 1) * 512], ob[:]
                )

            qk_tiles = {}
            for h in range(HPC):
                qk_tiles[h] = (
                    qkpool.tile([128, N], F32, tag="qk", name=f"tq{h}"),
                    qkpool.tile([128, N], F32, tag="qk", name=f"tk{h}"),
                )

            dveq = []  # deferrable combine-chain steps (DVE/gpsimd side)

            def combine(h, qc, o1, o2, defer):
                # Part 1 (immediate): stage the two rowsum rows at partition 0
                # (the custom-DVE approx reciprocal mis-reads partition-offset
                # inputs on HW) and evacuate psum, freeing the po banks fast.
                # Part 2 (deferrable into the next qc's groups, keeping the
                # DVE queue short for production evacs): 1/r via the fast
                # approx reciprocal, fold -lambda into the second half,
                # replicate across the 64 hd-rows on gpsimd, scale + add into
                # the head-group rows.
                rs = rpool.tile([1, 1024], F32, tag="rs", name=f"rs_{h}_{qc}")
                nc.vector.tensor_copy(rs[:, 0:512], o1[HD:HD + 1, :])
                nc.vector.tensor_copy(rs[:, 512:1024], o2[HD:HD + 1, :])
                o12 = cpool.tile([HD, 1024], F32, tag="o12", name=f"o12_{h}_{qc}")
                nc.vector.tensor_copy(o12[:, 0:512], o1[0:HD, :])
                nc.vector.tensor_copy(o12[:, 512:1024], o2[0:HD, :])
                rr = rpool.tile([1, 1024], F32, tag="rr", name=f"rr_{h}_{qc}")
                rep = tpool.tile([HD, 1024], F32, tag="repb", name=f"repb_{h}_{qc}")
                t12 = tpool.tile([HD, 1024], F32, tag="t12")

                def c1():
                    nc.vector.reciprocal_approx_fast(out=rr[:], in_=rs[:])
                    nc.vector.tensor_scalar_mul(rr[:, 512:1024], rr[:, 512:1024], negl[:])

                def c2():
                    nc.gpsimd.partition_broadcast(rep[:], rr[:])

                def c3():
                    nc.vector.tensor_mul(t12[:], o12[0:HD, :], rep[0:HD, 0:1024])
                    nc.vector.tensor_add(
                        o_t[h // 2][
                            (h % 2) * 64:(h % 2) * 64 + 64,
                            qc * 512:(qc + 1) * 512,
                        ],
                        t12[:, 0:512],
                        t12[:, 512:1024],
                    )

                if defer:
                    dveq.extend([c1, c2, c3])
                else:
                    c1()
                    c2()
                    c3()

            def w_sb_at(wi):
                kind, hh = wi
                return (w_q if kind == "q" else w_k)[hh]

            bkq = []  # FIFO of background PE step closures

            with (
                tc.tile_pool(name="slots", bufs=2, space="PSUM") as slots,
                tc.tile_pool(name="po", bufs=2, space="PSUM") as po,
            ):
                # pending attn@V pairs: emitted two groups late so the PE's
                # av matmuls trail the exp by two full groups of slack.
                pend = []

                def emit_av(h, o1, o2, kt, u):
                    nc.tensor.matmul(
                        o1[:],
                        lhsT=r(vaug[:, h, kt, :]),
                        rhs=r(u[:, 0:512]),
                        start=(kt == 0),
                        stop=(kt == NT - 1),
                    )
                    nc.tensor.matmul(
                        o2[:],
                        lhsT=r(vaug[:, h, kt, :]),
                        rhs=r(u[:, 512:1024]),
                        start=(kt == 0),
                        stop=(kt == NT - 1),
                    )

                def group(h, tq, tk, qc, kt, o1, o2):
                    sl = slots.tile([128, 1024], F32, tag="slot")
                    for term in range(2):
                        rb = term * 64
                        nc.tensor.matmul(
                            sl[:, term * 512:(term + 1) * 512],
                            lhsT=r(tk[rb:rb + 64, kt * 128:(kt + 1) * 128]),
                            rhs=r(tq[rb:rb + 64, qc * 512:(qc + 1) * 512]),
                            start=True,
                            stop=True,
                        )
                    u = upool.tile([128, 1024], F32R, tag="u")
                    nc.scalar.activation(u[:], sl[:], AF.Exp, scale=SCALE)
                    if len(pend) == 2:
                        emit_av(*pend.pop(0))
                    pend.append((h, o1, o2, kt, u))

                # -- head 0, qc 0: interleave the V/K production with the
                # first attention groups (each group g consumes key-block
                # kt=g, whose v/k tiles are produced a few groups ahead).
                tq0, tk0 = qk_tiles[0]
                for nt in range(4):
                    v_nt(nt)
                qk_piece(w_q[0], tq0, 0)
                qk_piece(w_k[0], tk0, 0)
                q0fill = [
                    [lambda: v_nt(4), lambda: v_nt(5)],
                    [lambda: qk_piece(w_k[0], tk0, 1)],
                    [lambda: v_nt(6), lambda: v_nt(7)],
                    [lambda: qk_piece(w_k[0], tk0, 2)],
                    [lambda: v_nt(8), lambda: v_nt(9)],
                    [lambda: v_nt(10), lambda: v_nt(11)],
                    [lambda: qk_piece(w_k[0], tk0, 3)],
                    [lambda: v_nt(12), lambda: v_nt(13)],
                    [lambda: v_nt(14), lambda: v_nt(15)],
                    [lambda: qk_piece(w_q[0], tq0, 1)],
                    [lambda: qk_piece(w_q[0], tq0, 2)],
                    [lambda: qk_piece(w_q[0], tq0, 3)],
                    [], [], [], [],
                ]

                for h in range(HPC):
                    tq, tk = qk_tiles[h]
                    if h == 1:
                        w_q[3] = load_wqk(3)
                        w_k[3] = load_wqk(7)
                    if h < 3:
                        nh = h + 1
                        tqn, tkn = qk_tiles[nh]
                        for qc in range(QC):
                            bkq += qk_steps(("q", nh), tqn, qc)
                            bkq += qk_steps(("k", nh), tkn, qc)
                    for qc in range(QC):
                        if h == 0 and qc == 1:
                            w_q[2] = load_wqk(2)
                            w_k[2] = load_wqk(6)
                        o1 = po.tile([HD + 1, 512], F32, tag="oacc")
                        o2 = po.tile([HD + 1, 512], F32, tag="oacc")
                        first = (h == 0 and qc == 0)
                        stride = max(1, ((QC - qc) * NT) // max(len(bkq), 1))
                        for kt in range(NT):
                            group(h, tq, tk, qc, kt, o1, o2)
                            if dveq:
                                dveq.pop(0)()
                            if first:
                                for f in q0fill[kt]:
                                    f()
                            elif bkq and kt % stride == stride - 1:
                                bkq.pop(0)()
                        while pend:
                            emit_av(*pend.pop(0))
                        combine(h, qc, o1, o2, defer=(h < 3))
                        if h == 3 and qc < QC - 1:
                            # output projection for this q-chunk (all heads'
                            # combines for it are now done)
                            for ot in range(8):
                                bkq.append(lambda nch=qc, ot=ot: proj_piece(nch, ot))
                    while bkq:
                        bkq.pop(0)()
                    while dveq:
                        dveq.pop(0)()

            # ---- tail: last proj chunk + sum-of-squares ----
            with tc.tile_pool(name="psS", bufs=1, space="PSUM") as psS:
                for ot in range(8):
                    proj_piece(3, ot)
                ssq_ps = psS.tile([1, N], F32, tag="ssqp")
                for c4 in range(QC):
                    for t in range(2):
                        sq = sqpool.tile([128, 512], F32R, tag="sq", bufs=1)
                        nc.scalar.activation(
                            sq[:], o_t[t][:, c4 * 512:(c4 + 1) * 512], AF.Square
                        )
                        nc.tensor.matmul(
                            ssq_ps[:, c4 * 512:(c4 + 1) * 512],
                            lhsT=r(ones128[:]),
                            rhs=r(sq[:]),
                            start=(t == 0),
                            stop=(t == 1),
                        )
                for c4 in range(QC):
                    ssq_sb = sqpool.tile([1, 512], F32, tag="ssqs", bufs=1)
                    nc.vector.tensor_copy(ssq_sb[:], ssq_ps[:, c4 * 512:(c4 + 1) * 512])
                    nc.sync.dma_start(ssq[:, c4 * 512:(c4 + 1) * 512], ssq_sb[:])
            if debug:
                d_ot0 = nc.dram_tensor("d_ot0", [128, N], F32, kind="ExternalOutput").ap()
                d_ot1 = nc.dram_tensor("d_ot1", [128, N], F32, kind="ExternalOutput").ap()
                d_va = nc.dram_tensor(
                    "d_va", [128, HPC * NT * (HD + 1)], F32, kind="ExternalOutput"
                ).ap()
                nc.sync.dma_start(d_ot0[:], o_t[0].bitcast(F32)[:])
                nc.sync.dma_start(d_ot1[:], o_t[1].bitcast(F32)[:])
                nc.sync.dma_start(
                    d_va[:], vaug.rearrange("p a b c -> p (a b c)")[:]
                )
    return nc


_CACHE = {}


def get_nc():
    if "nc" not in _CACHE:
        nc = bacc.Bacc(
            "TRN2", target_bir_lowering=False, debug=False, enable_asserts=False
        )
        build_program(nc)
        nc.compile()
        nc.m = get_hw_module(nc.m)
        _CACHE["nc"] = nc
    return _CACHE["nc"]


def make_in_maps(x, qkv_w, proj_w, lambda_q1, lambda_k1, lambda_q2, lambda_k2):
    x = np.asarray(x, np.float32)
    qkv_w = np.asarray(qkv_w, np.float32)
    proj_w = np.asarray(proj_w, np.float32)
    lamv = np.concatenate(
        [np.asarray(a, np.float32) for a in (lambda_q1, lambda_k1, lambda_q2, lambda_k2)]
    )[None, :]
    in_maps = []
    for core in range(8):
        b, hg = core // 4, core % 4
        h0 = hg * HPC
        rows = []
        for h in range(h0, h0 + HPC):
            rows.append(qkv_w[0 * DIM + h * HD:0 * DIM + (h + 1) * HD])
            rows.append(qkv_w[1 * DIM + h * HD:1 * DIM + (h + 1) * HD])
        for h in range(h0, h0 + HPC):
            rows.append(qkv_w[2 * DIM + h * HD:2 * DIM + (h + 1) * HD])
            rows.append(qkv_w[3 * DIM + h * HD:3 * DIM + (h + 1) * HD])
        wqk_np = np.ascontiguousarray(np.concatenate(rows, 0).T)
        wv_np = np.ascontiguousarray(
            np.concatenate(
                [qkv_w[4 * DIM + h * HD:4 * DIM + (h + 1) * HD] for h in range(h0, h0 + HPC)],
                0,
            ).T
        )
        wp_np = np.ascontiguousarray(proj_w[:, h0 * HD:(h0 + HPC) * HD].T)
        in_maps.append(
            {
                "xt": np.ascontiguousarray(x[b].T),
                "wqk": wqk_np,
                "wv": wv_np,
                "wp": wp_np,
                "lam": np.ascontiguousarray(lamv),
            }
        )
    return in_maps


def combine(results, proj_b):
    proj_b = np.asarray(proj_b, np.float32)
    y = np.empty((B, N, DIM), np.float32)
    for b in range(B):
        acc = np.zeros((DIM, N), np.float64)
        sq = np.zeros(N, np.float64)
        for g in range(4):
            rr = results[b * 4 + g]
            acc += rr["out"].astype(np.float64)
            sq += rr["ssq"][0].astype(np.float64)
        s = 0.2 / np.sqrt(sq / DIM + EPS)
        y[b] = (acc.T * s[:, None] + proj_b).astype(np.float32)
    return y


def kernel(x, qkv_w, proj_w, proj_b, lambda_q1, lambda_k1, lambda_q2, lambda_k2):
    nc = get_nc()
    in_maps = make_in_maps(
        x, qkv_w, proj_w, lambda_q1, lambda_k1, lambda_q2, lambda_k2
    )
    res = bass_utils.run_bass_kernel_spmd(nc, in_maps, core_ids=list(range(8)))
    return combine(res.results, proj_b)
